# revision 10
# baseline (speedup 1.0000x reference)
"""DNDF tree (soft decision tree / dense MoE) kernel for Trainium2.

Full computation (reference):
    dprob  = sigmoid(x @ Wd.T + bd)                 [B, 63]
    routing[b, l] = prod_d (pos ? dprob[idx] : 1 - dprob[idx])   [B, 64]
    leaves = softmax(einsum('bi,loi->blo', x, Wl) + bl, axis=-1) [B, 64, O]
    out    = einsum('bl,blo->bo', routing, leaves)  [B, O]

Sharding: expert-parallel over the 64 leaves: core c owns leaves
8c..8c+7, computes partial = sum_{l in core} routing[:, l] * leaves[:, l, :]
over the FULL batch; the host sums the 8 per-core partials.

Routing trick (no gathers): with z = x@Wd.T + bd,
    log p      = -softplus(-z),   log(1-p) = -softplus(z)
    log routing[b,l] = -( softplus(-z) @ A + softplus(z) @ B )[b, l]
where A[n,l]=1 iff leaf l visits node n on the sigmoid branch and
B[n,l]=1 iff on the (1-sigmoid) branch.  Two small matmuls + exp.
softplus(t) = Ln(Exp(t)+1) — computed with the one ACT table set that
has both exp and ln (Softplus has no table on cayman).

float16 matmuls: 10-bit mantissa keeps end-to-end error ~3e-4 while
streaming 1 col/cycle on the PE with FWL weight loads.  (fp8 DoubleRow
was evaluated and rejected: e4m3 quantization alone gives rel_err
2.3e-2 > the 2e-2 gate, and any 2-slot correction scheme lands at f16
cost with worse per-slot throughput.)

Schedule (from perfetto analysis of the 516us baseline):
  - PE warm-up runs on a memset tile (no DMA dependency) so the HAM
    clock-gate reaches 8/8 at ~1us and the routing matmuls never run at
    the 1.2-1.6 GHz cold rate (was ~16us of cold tax + 11.5us of gaps).
  - x is DMA'd batch-block-major and interleaved with expert 0's
    weights so each 512-sample block's routing (z matmuls + softplus +
    leaf-product) and expert-0 tiles start as soon as that block lands.
  - Output writeback uses full 4 KiB DRAM lines; the last batch tile
    fans out over 16 queues to cut the drain tail.
"""

import numpy as np
import sys

for _p in ("/opt/trn_rl_repo", "/opt/pypackages"):
    if _p not in sys.path:
        sys.path.append(_p)

import concourse.bass as bass  # noqa: E402,F401
import concourse.bacc as bacc  # noqa: E402
import concourse.tile as tile  # noqa: E402
from concourse import mybir  # noqa: E402
from concourse.bass_utils import run_bass_kernel_spmd  # noqa: E402

TREE_DEPTH = 6
IN_F = 1024
OUT_F = 1024
BATCH = 2048
N_LEAVES = 64
N_NODES = 63
N_CORES = 8
E = N_LEAVES // N_CORES  # experts per core = 8
IC = IN_F // 128  # contraction chunks = 8
BT = BATCH // 128  # batch tiles = 16
NBC = BATCH // 512  # 512-sample batch blocks = 4
F32 = mybir.dt.float32
BF16 = mybir.dt.bfloat16
F16 = mybir.dt.float16
AF = mybir.ActivationFunctionType
ALU = mybir.AluOpType

MMDT = "f16"
WARM_N = 12  # warm-up matmuls bridging engine start -> first x block


def _tree_routes(depth):
    n_leaves = 2**depth
    idx = np.zeros((n_leaves, depth), dtype=np.int32)
    pos = np.zeros((n_leaves, depth), dtype=bool)
    for leaf in range(n_leaves):
        node, index = 0, leaf
        for d in range(depth):
            idx[leaf, d] = node
            pos[leaf, d] = index % 2 == 0
            node = node * 2 + 1 + index % 2
            index >>= 1
    return idx, pos


ROUTE_IDX, ROUTE_POS = _tree_routes(TREE_DEPTH)


def _selection_matrices():
    """A[n, l]=1 iff leaf l visits node n with the sigmoid branch; B for 1-sigmoid."""
    A = np.zeros((N_NODES, N_LEAVES), dtype=np.float32)
    B = np.zeros((N_NODES, N_LEAVES), dtype=np.float32)
    for leaf in range(N_LEAVES):
        for d in range(TREE_DEPTH):
            n = ROUTE_IDX[leaf, d]
            if ROUTE_POS[leaf, d]:
                A[n, leaf] = 1.0
            else:
                B[n, leaf] = 1.0
    return A, B


A_FULL, B_FULL = _selection_matrices()


def build_nc(add_bd: bool, add_bl: bool, mmdt: str = MMDT):
    """Build + compile the single-core Bass program (same NEFF on all cores)."""
    from contextlib import ExitStack

    MDT = {"bf16": BF16, "f16": F16}[mmdt]

    nc = bacc.Bacc("TRN2", target_bir_lowering=False, debug=False)

    xT_d = nc.dram_tensor("xT", [IN_F, BATCH], MDT, kind="ExternalInput")
    wdT_d = nc.dram_tensor("wdT", [IN_F, N_NODES], MDT, kind="ExternalInput")
    wl_d = nc.dram_tensor("wl", [E, IN_F, OUT_F], MDT, kind="ExternalInput")
    a_d = nc.dram_tensor("amat", [N_NODES, E], F32, kind="ExternalInput")
    b_d = nc.dram_tensor("bmat", [N_NODES, E], F32, kind="ExternalInput")
    bd_d = bl_d = None
    if add_bd:
        bd_d = nc.dram_tensor("bd", [1, N_NODES], MDT, kind="ExternalInput")
    if add_bl:
        bl_d = nc.dram_tensor("bl", [E, OUT_F], MDT, kind="ExternalInput")
    out_d = nc.dram_tensor("out", [BATCH, OUT_F], F32, kind="ExternalOutput")

    mm = lambda out, lhsT, rhs, start, stop: nc.tensor.matmul(  # noqa: E731
        out, lhsT, rhs, start=start, stop=stop
    )

    with ExitStack() as ctx:
        tc = ctx.enter_context(tile.TileContext(nc))
        consts = ctx.enter_context(tc.tile_pool(name="consts", bufs=1))
        xp = ctx.enter_context(tc.tile_pool(name="xp", bufs=1))
        wp = ctx.enter_context(tc.tile_pool(name="wp", bufs=3))
        accp = ctx.enter_context(tc.tile_pool(name="accp", bufs=BT))
        expp = ctx.enter_context(tc.tile_pool(name="expp", bufs=3))
        rpool = ctx.enter_context(tc.tile_pool(name="rpool", bufs=17))
        spp = ctx.enter_context(tc.tile_pool(name="spp", bufs=1))
        smallp = ctx.enter_context(tc.tile_pool(name="smallp", bufs=6))
        zps = ctx.enter_context(tc.tile_pool(name="zps", bufs=3, space="PSUM"))
        rps = ctx.enter_context(tc.tile_pool(name="rps", bufs=2, space="PSUM"))
        lps = ctx.enter_context(tc.tile_pool(name="lps", bufs=3, space="PSUM"))

        # ---- PE warm-up with no DMA dependency (memset-sourced tile).
        # Starts as soon as the engines are live (~8us), flips the HAM clock
        # gate to 8/8 before the first real matmul, and bridges the DMA
        # spin-up window so routing runs at the warm rate.
        warm_t = consts.tile([128, 512], MDT)
        nc.vector.memset(warm_t, 0.05)
        warm_ps = zps.tile([128, 512], F32, tag="z")
        for i in range(WARM_N):
            mm(warm_ps, warm_t[:, 0:128], warm_t, start=(i == 0), stop=(i == WARM_N - 1))
        junk_t = smallp.tile([128, 1], F32, tag="junk")
        nc.vector.reduce_max(junk_t, warm_ps, axis=mybir.AxisListType.X)

        # ---- streamed inputs.  x batch-block 0 goes out first, split 16
        # ways so every queue carries one 64 KiB piece and the block lands
        # ~3us after queue spin-up.
        x_t = xp.tile([128, IC, BATCH], MDT, tag="x", name="x0")

        def dma_x_block(k, nsplit=1):
            for c in range(IC):
                for s in range(nsplit):
                    r0 = c * 128 + s * (128 // nsplit)
                    r1 = r0 + 128 // nsplit
                    p0 = s * (128 // nsplit)
                    nc.sync.dma_start(
                        out=x_t[p0 : p0 + 128 // nsplit, c, k * 512 : (k + 1) * 512],
                        in_=xT_d[r0:r1, k * 512 : (k + 1) * 512],
                    )

        dma_x_block(0, nsplit=2)  # queues 0-15: one 64 KiB piece each

        wd_t = consts.tile([128, IC, N_NODES], MDT)  # tiny, queued 2nd
        for c in range(IC):
            nc.sync.dma_start(out=wd_t[:, c, :], in_=wdT_d[c * 128 : (c + 1) * 128, :])
        a_t = consts.tile([N_NODES, E], F32)
        nc.sync.dma_start(out=a_t, in_=a_d[:])
        b_t = consts.tile([N_NODES, E], F32)
        nc.sync.dma_start(out=b_t, in_=b_d[:])
        ones_t = bd_t = None
        if add_bd or add_bl:
            ones_t = consts.tile([1, 512], MDT)
            nc.vector.memset(ones_t, 1.0)
        if add_bd:
            bd_t = consts.tile([1, N_NODES], MDT)
            nc.sync.dma_start(out=bd_t, in_=bd_d[:])

        def dma_w(e, ocs=(0, 1), nsplit=1, w_t=None):
            if w_t is None:
                w_t = wp.tile([128, IC, OUT_F], MDT, tag="w", name=f"w_{e}")
            for oc in ocs:
                for c in range(IC):
                    for s in range(nsplit):
                        r0 = c * 128 + s * (128 // nsplit)
                        p0 = s * (128 // nsplit)
                        nc.sync.dma_start(
                            out=w_t[
                                p0 : p0 + 128 // nsplit, c,
                                oc * 512 : (oc + 1) * 512,
                            ],
                            in_=wl_d[
                                e, r0 : r0 + 128 // nsplit,
                                oc * 512 : (oc + 1) * 512,
                            ],
                        )
            return w_t

        # expert 0's first weight half, 16-way split right behind x block 0
        w0_t = dma_w(0, ocs=(0,), nsplit=2)
        dma_x_block(1)
        dma_w(0, ocs=(1,), nsplit=2, w_t=w0_t)
        dma_x_block(2)
        dma_x_block(3)
        w1_t = dma_w(1)  # prefetch: in queues before the expert stream begins

        sp_pos = spp.tile([N_NODES, BATCH], F32, tag="sp_pos")
        sp_neg = spp.tile([N_NODES, BATCH], F32, tag="sp_neg")
        r_ts = {}

        def routing_wave(k):
            """z matmuls + softplus for batch block k, then leaf products
            (r) for its four 128-sample tiles."""
            sl = slice(k * 512, (k + 1) * 512)
            z_ps = zps.tile([N_NODES, 512], F32, tag="z")
            for c in range(IC):
                mm(
                    z_ps,
                    wd_t[:, c, :],
                    x_t[:, c, sl],
                    start=(c == 0),
                    stop=(c == IC - 1 and not add_bd),
                )
            if add_bd:
                mm(z_ps, bd_t[0:1, :], ones_t[0:1, :], start=False, stop=True)
            # softplus(z) and softplus(-z) via exp + ln(x+1)
            nc.scalar.activation(sp_pos[:, sl], z_ps, AF.Exp, scale=1.0)
            nc.scalar.activation(sp_pos[:, sl], sp_pos[:, sl], AF.Ln, bias=1.0)
            # softplus(-z) = softplus(z) - z (exact); DVE is idle here
            nc.vector.tensor_sub(sp_neg[:, sl], sp_pos[:, sl], z_ps)
            for bt in range(4 * k, 4 * k + 4):
                r_ps = rps.tile([128, E], F32, tag="r_ps")
                nc.tensor.matmul(
                    r_ps,
                    sp_neg[:, bt * 128 : (bt + 1) * 128],
                    a_t,
                    start=True,
                    stop=False,
                )
                nc.tensor.matmul(
                    r_ps,
                    sp_pos[:, bt * 128 : (bt + 1) * 128],
                    b_t,
                    start=False,
                    stop=True,
                )
                r_t = rpool.tile([128, E], F32, tag="r", name=f"r_{bt}")
                nc.scalar.activation(r_t, r_ps, AF.Exp, scale=-1.0)
                r_ts[bt] = r_t

        s0_ts = {}

        def expert_half(e, bt, oc, w_t, accs, ble_t=None):
            """Matmul group + exp eviction for one [128, 512] logit tile.
            Returns the per-row exp sum accumulator."""
            target = accs[bt] if e == 0 else expert_half.cur_exp
            lp = lps.tile([128, 512], F32, tag="lp")
            for c in range(IC):
                mm(
                    lp,
                    x_t[:, c, bt * 128 : (bt + 1) * 128],
                    w_t[:, c, oc * 512 : (oc + 1) * 512],
                    start=(c == 0),
                    stop=(c == IC - 1 and not add_bl),
                )
            if add_bl:
                mm(
                    lp,
                    ones_t[0:1, 0:128],
                    ble_t[0:1, oc * 512 : (oc + 1) * 512],
                    start=False,
                    stop=True,
                )
            sh = smallp.tile(
                [128, 1], F32,
                tag=(f"s0h{oc}" if e == 0 else f"sh{oc}"),
                bufs=(BT + 2 if e == 0 else 6),
                name=f"sh_{e == 0}_{oc}",
            )
            nc.scalar.activation(
                target[:, oc * 512 : (oc + 1) * 512], lp, AF.Exp, accum_out=sh
            )
            return sh

        def expert_tail(e, bt, accs, s_hs):
            """Softmax scale + routing-weighted accumulate for one batch tile."""
            if e == 1:
                # deferred scale of expert 0's parked (unscaled-exp) tile
                s0s = smallp.tile([128, 1], F32, tag="s0s")
                nc.vector.tensor_add(s0s, s0_ts[bt][0], s0_ts[bt][1])
                d_t = smallp.tile([128, 1], F32, tag="d")
                nc.vector.reciprocal(d_t, s0s)
                d2_t = smallp.tile([128, 1], F32, tag="d2")
                nc.vector.tensor_scalar_mul(d2_t, d_t, r_ts[bt][:, 0:1])
                nc.vector.tensor_scalar_mul(accs[bt], accs[bt], d2_t)
            exp_t = expert_tail.cur_exp
            s_t = smallp.tile([128, 1], F32, tag="s")
            nc.vector.tensor_add(s_t, s_hs[0], s_hs[1])
            sc_t = smallp.tile([128, 1], F32, tag="sc")
            nc.vector.reciprocal(sc_t, s_t)
            sc2_t = smallp.tile([128, 1], F32, tag="sc2")
            nc.vector.tensor_scalar_mul(sc2_t, sc_t, r_ts[bt][:, e : e + 1])
            if e == E - 1:
                # split the combine so the writeback chain is short, then
                # write full 4 KiB DRAM lines; the final tile fans out over
                # 16 queues to cut the drain tail.
                for oc in range(2):
                    osl = slice(oc * 512, (oc + 1) * 512)
                    nc.vector.scalar_tensor_tensor(
                        accs[bt][:, osl], exp_t[:, osl], sc2_t, accs[bt][:, osl],
                        op0=ALU.mult, op1=ALU.add,
                    )
                # 8 pieces of [16, 1024] = 64 KiB each: full 4 KiB DRAM
                # lines, small enough that no queue is monopolized >3us
                # while the final tiles drain.
                r0 = bt * 128
                for q in range(8):
                    nc.sync.dma_start(
                        out=out_d[r0 + q * 16 : r0 + (q + 1) * 16, :],
                        in_=accs[bt][q * 16 : (q + 1) * 16, :],
                    )
            else:
                nc.vector.scalar_tensor_tensor(
                    accs[bt], exp_t, sc2_t, accs[bt], op0=ALU.mult, op1=ALU.add
                )

        accs = [
            accp.tile([128, OUT_F], F32, tag="acc", name=f"acc_{bt}")
            for bt in range(BT)
        ]

        # ---- emission order: per-block waves while x streams in ----
        for k in range(NBC):
            routing_wave(k)
            for bt in range(4 * k, 4 * k + 4):
                sh = expert_half(0, bt, 0, w0_t, accs)
                s0_ts[bt] = [sh]
        # expert 0's second half-columns (oc1 weights landed with block 1)
        for bt in range(BT):
            sh = expert_half(0, bt, 1, w0_t, accs)
            s0_ts[bt].append(sh)

        w_tiles = {1: w1_t}
        for e in range(1, E):
            if e + 1 < E:
                # prefetch next expert's weights so its transfers sit in the
                # queues a full expert (~55us of PE work) ahead of use
                w_tiles[e + 1] = dma_w(e + 1)
            w_t = w_tiles.pop(e)
            ble_t = None
            if add_bl:
                ble_t = smallp.tile([1, OUT_F], MDT, tag="ble", bufs=2)
                nc.sync.dma_start(out=ble_t, in_=bl_d[e : e + 1, :])
            for bt in range(BT):
                exp_t = expp.tile([128, OUT_F], F32, tag="exp", name="exp_t")
                expert_half.cur_exp = exp_t
                expert_tail.cur_exp = exp_t
                s_hs = [
                    expert_half(e, bt, 0, w_t, accs, ble_t),
                    expert_half(e, bt, 1, w_t, accs, ble_t),
                ]
                expert_tail(e, bt, accs, s_hs)

    nc.compile()
    return nc


def make_core_inputs(x, Wd, bd, Wl, bl, core, add_bd, add_bl, mmdt: str = MMDT):
    import ml_dtypes

    ndt = {"bf16": ml_dtypes.bfloat16, "f16": np.float16}[mmdt]
    lo, hi = core * E, (core + 1) * E
    m = {
        "xT": np.ascontiguousarray(x.T.astype(ndt)),
        "wdT": np.ascontiguousarray(Wd.T.astype(ndt)),
        "wl": np.ascontiguousarray(Wl[lo:hi].transpose(0, 2, 1).astype(ndt)),
        "amat": np.ascontiguousarray(A_FULL[:, lo:hi]),
        "bmat": np.ascontiguousarray(B_FULL[:, lo:hi]),
    }
    if add_bd:
        m["bd"] = np.ascontiguousarray(bd.astype(ndt)).reshape(1, N_NODES)
    if add_bl:
        m["bl"] = np.ascontiguousarray(bl[lo:hi].astype(ndt))
    return m


_NC_CACHE = {}


def _get_nc(add_bd, add_bl, mmdt: str = MMDT):
    key = (add_bd, add_bl, mmdt)
    if key not in _NC_CACHE:
        _NC_CACHE[key] = build_nc(add_bd, add_bl, mmdt)
    return _NC_CACHE[key]


def run_spmd(x, Wd, bd, Wl, bl, trace=False, mmdt: str = MMDT):
    add_bd = bool(np.any(bd))
    add_bl = bool(np.any(bl))
    nc = _get_nc(add_bd, add_bl, mmdt)
    in_maps = [
        make_core_inputs(x, Wd, bd, Wl, bl, c, add_bd, add_bl, mmdt)
        for c in range(N_CORES)
    ]
    res = run_bass_kernel_spmd(nc, in_maps, core_ids=list(range(N_CORES)), trace=trace)
    partials = np.stack([r["out"] for r in res.results], axis=0)
    out = partials.sum(axis=0, dtype=np.float64).astype(np.float32)
    return out, res


def kernel(x, Wd, bd, Wl, bl):
    out, _ = run_spmd(
        np.asarray(x), np.asarray(Wd), np.asarray(bd), np.asarray(Wl), np.asarray(bl)
    )
    return out


# revision 13
# speedup vs baseline: 1.0470x; 1.0470x over previous
"""DNDF tree (soft decision tree / dense MoE) kernel for Trainium2.

Full computation (reference):
    dprob  = sigmoid(x @ Wd.T + bd)                 [B, 63]
    routing[b, l] = prod_d (pos ? dprob[idx] : 1 - dprob[idx])   [B, 64]
    leaves = softmax(einsum('bi,loi->blo', x, Wl) + bl, axis=-1) [B, 64, O]
    out    = einsum('bl,blo->bo', routing, leaves)  [B, O]

Sharding: expert-parallel over the 64 leaves: core c owns leaves
8c..8c+7, computes partial = sum_{l in core} routing[:, l] * leaves[:, l, :]
over the FULL batch; the host sums the 8 per-core partials.

Routing trick (no gathers): with z = x@Wd.T + bd,
    log p      = -softplus(-z),   log(1-p) = -softplus(z)
    log routing[b,l] = -( softplus(-z) @ A + softplus(z) @ B )[b, l]
where A[n,l]=1 iff leaf l visits node n on the sigmoid branch and
B[n,l]=1 iff on the (1-sigmoid) branch.  Two small matmuls + exp.
softplus(t) = Ln(Exp(t)+1) — computed with the one ACT table set that
has both exp and ln (Softplus has no table on cayman).

float16 matmuls: 10-bit mantissa keeps end-to-end error ~3e-4 while
streaming 1 col/cycle on the PE with FWL weight loads.  (fp8 DoubleRow
was evaluated and rejected: e4m3 quantization alone gives rel_err
2.3e-2 > the 2e-2 gate, and any 2-slot correction scheme lands at f16
cost with worse per-slot throughput.)

Schedule (from perfetto analysis of the 516us baseline):
  - PE warm-up runs on a memset tile (no DMA dependency) so the HAM
    clock-gate reaches 8/8 at ~1us and the routing matmuls never run at
    the 1.2-1.6 GHz cold rate (was ~16us of cold tax + 11.5us of gaps).
  - x is DMA'd batch-block-major and interleaved with expert 0's
    weights so each 512-sample block's routing (z matmuls + softplus +
    leaf-product) and expert-0 tiles start as soon as that block lands.
  - Output writeback uses full 4 KiB DRAM lines; the last batch tile
    fans out over 16 queues to cut the drain tail.
"""

import numpy as np
import sys

for _p in ("/opt/trn_rl_repo", "/opt/pypackages"):
    if _p not in sys.path:
        sys.path.append(_p)

import concourse.bass as bass  # noqa: E402,F401
import concourse.bacc as bacc  # noqa: E402
import concourse.tile as tile  # noqa: E402
from concourse import mybir  # noqa: E402
from concourse.bass_utils import run_bass_kernel_spmd  # noqa: E402

TREE_DEPTH = 6
IN_F = 1024
OUT_F = 1024
BATCH = 2048
N_LEAVES = 64
N_NODES = 63
N_CORES = 8
E = N_LEAVES // N_CORES  # experts per core = 8
IC = IN_F // 128  # contraction chunks = 8
BT = BATCH // 128  # batch tiles = 16
NBC = BATCH // 512  # 512-sample batch blocks = 4
F32 = mybir.dt.float32
BF16 = mybir.dt.bfloat16
F16 = mybir.dt.float16
AF = mybir.ActivationFunctionType
ALU = mybir.AluOpType

MMDT = "f16"
WARM_N = 32  # warm-up matmuls bridging engine start -> first x block (~19us)


def _tree_routes(depth):
    n_leaves = 2**depth
    idx = np.zeros((n_leaves, depth), dtype=np.int32)
    pos = np.zeros((n_leaves, depth), dtype=bool)
    for leaf in range(n_leaves):
        node, index = 0, leaf
        for d in range(depth):
            idx[leaf, d] = node
            pos[leaf, d] = index % 2 == 0
            node = node * 2 + 1 + index % 2
            index >>= 1
    return idx, pos


ROUTE_IDX, ROUTE_POS = _tree_routes(TREE_DEPTH)


def _selection_matrices():
    """A[n, l]=1 iff leaf l visits node n with the sigmoid branch; B for 1-sigmoid."""
    A = np.zeros((N_NODES, N_LEAVES), dtype=np.float32)
    B = np.zeros((N_NODES, N_LEAVES), dtype=np.float32)
    for leaf in range(N_LEAVES):
        for d in range(TREE_DEPTH):
            n = ROUTE_IDX[leaf, d]
            if ROUTE_POS[leaf, d]:
                A[n, leaf] = 1.0
            else:
                B[n, leaf] = 1.0
    return A, B


A_FULL, B_FULL = _selection_matrices()


def build_nc(add_bd: bool, add_bl: bool, mmdt: str = MMDT):
    """Build + compile the single-core Bass program (same NEFF on all cores)."""
    from contextlib import ExitStack

    MDT = {"bf16": BF16, "f16": F16}[mmdt]

    nc = bacc.Bacc("TRN2", target_bir_lowering=False, debug=False)

    xT_d = nc.dram_tensor("xT", [IN_F, BATCH], MDT, kind="ExternalInput")
    wdT_d = nc.dram_tensor("wdT", [IN_F, N_NODES], MDT, kind="ExternalInput")
    wl_d = nc.dram_tensor("wl", [E, IN_F, OUT_F], MDT, kind="ExternalInput")
    a_d = nc.dram_tensor("amat", [N_NODES, E], F32, kind="ExternalInput")
    b_d = nc.dram_tensor("bmat", [N_NODES, E], F32, kind="ExternalInput")
    bd_d = bl_d = None
    if add_bd:
        bd_d = nc.dram_tensor("bd", [1, N_NODES], MDT, kind="ExternalInput")
    if add_bl:
        bl_d = nc.dram_tensor("bl", [E, OUT_F], MDT, kind="ExternalInput")
    out_d = nc.dram_tensor("out", [BATCH, OUT_F], F32, kind="ExternalOutput")

    mm = lambda out, lhsT, rhs, start, stop: nc.tensor.matmul(  # noqa: E731
        out, lhsT, rhs, start=start, stop=stop
    )

    with ExitStack() as ctx:
        tc = ctx.enter_context(tile.TileContext(nc))
        consts = ctx.enter_context(tc.tile_pool(name="consts", bufs=1))
        xp = ctx.enter_context(tc.tile_pool(name="xp", bufs=1))
        wp = ctx.enter_context(tc.tile_pool(name="wp", bufs=3))
        accp = ctx.enter_context(tc.tile_pool(name="accp", bufs=BT))
        expp = ctx.enter_context(tc.tile_pool(name="expp", bufs=3))
        rpool = ctx.enter_context(tc.tile_pool(name="rpool", bufs=17))
        spp = ctx.enter_context(tc.tile_pool(name="spp", bufs=1))
        smallp = ctx.enter_context(tc.tile_pool(name="smallp", bufs=6))
        zps = ctx.enter_context(tc.tile_pool(name="zps", bufs=3, space="PSUM"))
        rps = ctx.enter_context(tc.tile_pool(name="rps", bufs=2, space="PSUM"))
        lps = ctx.enter_context(tc.tile_pool(name="lps", bufs=3, space="PSUM"))

        # ---- PE warm-up with no DMA dependency (memset-sourced tile).
        # Starts as soon as the engines are live (~8us), flips the HAM clock
        # gate to 8/8 before the first real matmul, and bridges the DMA
        # spin-up window so routing runs at the warm rate.
        warm_t = consts.tile([128, 512], MDT)
        nc.vector.memset(warm_t, 0.05)
        warm_ps = zps.tile([128, 512], F32, tag="z")
        for i in range(WARM_N):
            mm(warm_ps, warm_t[:, 0:128], warm_t, start=(i == 0), stop=(i == WARM_N - 1))
        junk_t = smallp.tile([128, 1], F32, tag="junk")
        nc.vector.reduce_max(junk_t, warm_ps, axis=mybir.AxisListType.X)

        # ---- streamed inputs.  x batch-block 0 goes out first, split 16
        # ways so every queue carries one 64 KiB piece and the block lands
        # ~3us after queue spin-up.
        x_t = xp.tile([128, IC, BATCH], MDT, tag="x", name="x0")

        def dma_x_block(k, nsplit=1):
            for c in range(IC):
                for s in range(nsplit):
                    r0 = c * 128 + s * (128 // nsplit)
                    r1 = r0 + 128 // nsplit
                    p0 = s * (128 // nsplit)
                    nc.sync.dma_start(
                        out=x_t[p0 : p0 + 128 // nsplit, c, k * 512 : (k + 1) * 512],
                        in_=xT_d[r0:r1, k * 512 : (k + 1) * 512],
                    )

        dma_x_block(0, nsplit=2)  # queues 0-15: one 64 KiB piece each

        wd_t = consts.tile([128, IC, N_NODES], MDT)  # tiny, queued 2nd
        for c in range(IC):
            nc.sync.dma_start(out=wd_t[:, c, :], in_=wdT_d[c * 128 : (c + 1) * 128, :])
        a_t = consts.tile([N_NODES, E], F32)
        nc.sync.dma_start(out=a_t, in_=a_d[:])
        b_t = consts.tile([N_NODES, E], F32)
        nc.sync.dma_start(out=b_t, in_=b_d[:])
        ones_t = bd_t = None
        if add_bd or add_bl:
            ones_t = consts.tile([1, 512], MDT)
            nc.vector.memset(ones_t, 1.0)
        if add_bd:
            bd_t = consts.tile([1, N_NODES], MDT)
            nc.sync.dma_start(out=bd_t, in_=bd_d[:])

        def dma_w(e, ocs=(0, 1), nsplit=1, w_t=None):
            if w_t is None:
                w_t = wp.tile([128, IC, OUT_F], MDT, tag="w", name=f"w_{e}")
            for oc in ocs:
                for c in range(IC):
                    for s in range(nsplit):
                        r0 = c * 128 + s * (128 // nsplit)
                        p0 = s * (128 // nsplit)
                        nc.sync.dma_start(
                            out=w_t[
                                p0 : p0 + 128 // nsplit, c,
                                oc * 512 : (oc + 1) * 512,
                            ],
                            in_=wl_d[
                                e, r0 : r0 + 128 // nsplit,
                                oc * 512 : (oc + 1) * 512,
                            ],
                        )
            return w_t

        # expert 0's weights, 16-way split per half, right behind x block 0
        w0_t = dma_w(0, ocs=(0,), nsplit=2)
        dma_w(0, ocs=(1,), nsplit=2, w_t=w0_t)
        dma_x_block(1)
        dma_x_block(2)
        dma_x_block(3)
        w1_t = dma_w(1)  # prefetch: in queues before the expert stream begins

        sp_pos = spp.tile([N_NODES, BATCH], F32, tag="sp_pos")
        sp_neg = spp.tile([N_NODES, BATCH], F32, tag="sp_neg")
        r_ts = {}

        def routing_wave(k):
            """z matmuls + softplus for batch block k, then leaf products
            (r) for its four 128-sample tiles."""
            sl = slice(k * 512, (k + 1) * 512)
            z_ps = zps.tile([N_NODES, 512], F32, tag="z")
            for c in range(IC):
                mm(
                    z_ps,
                    wd_t[:, c, :],
                    x_t[:, c, sl],
                    start=(c == 0),
                    stop=(c == IC - 1 and not add_bd),
                )
            if add_bd:
                mm(z_ps, bd_t[0:1, :], ones_t[0:1, :], start=False, stop=True)
            # softplus(z) and softplus(-z) via exp + ln(x+1)
            nc.scalar.activation(sp_pos[:, sl], z_ps, AF.Exp, scale=1.0)
            nc.scalar.activation(sp_pos[:, sl], sp_pos[:, sl], AF.Ln, bias=1.0)
            # softplus(-z) = softplus(z) - z (exact); DVE is idle here
            nc.vector.tensor_sub(sp_neg[:, sl], sp_pos[:, sl], z_ps)
            for bt in range(4 * k, 4 * k + 4):
                r_ps = rps.tile([128, E], F32, tag="r_ps")
                nc.tensor.matmul(
                    r_ps,
                    sp_neg[:, bt * 128 : (bt + 1) * 128],
                    a_t,
                    start=True,
                    stop=False,
                )
                nc.tensor.matmul(
                    r_ps,
                    sp_pos[:, bt * 128 : (bt + 1) * 128],
                    b_t,
                    start=False,
                    stop=True,
                )
                r_t = rpool.tile([128, E], F32, tag="r", name=f"r_{bt}")
                nc.scalar.activation(r_t, r_ps, AF.Exp, scale=-1.0)
                r_ts[bt] = r_t

        accs = [
            accp.tile([128, OUT_F], F32, tag="acc", name=f"acc_{bt}")
            for bt in range(BT)
        ]

        def expert_bt(e, bt, w_t, ble_t=None):
            """One batch tile of one expert: 16 matmuls, exp eviction with
            row-sum accumulators, softmax scale, routing-weighted combine.
            Expert 0 initializes the accumulator (mul), others add into it —
            every expert has the same engine mix, so no engine spikes."""
            exp_t = expp.tile([128, OUT_F], F32, tag="exp", name="exp_t")
            s_hs = []
            for oc in range(2):
                lp = lps.tile([128, 512], F32, tag="lp")
                for c in range(IC):
                    mm(
                        lp,
                        x_t[:, c, bt * 128 : (bt + 1) * 128],
                        w_t[:, c, oc * 512 : (oc + 1) * 512],
                        start=(c == 0),
                        stop=(c == IC - 1 and not add_bl),
                    )
                if add_bl:
                    mm(
                        lp,
                        ones_t[0:1, 0:128],
                        ble_t[0:1, oc * 512 : (oc + 1) * 512],
                        start=False,
                        stop=True,
                    )
                sh = smallp.tile([128, 1], F32, tag=f"sh{oc}", name=f"sh{oc}")
                nc.scalar.activation(
                    exp_t[:, oc * 512 : (oc + 1) * 512], lp, AF.Exp, accum_out=sh
                )
                s_hs.append(sh)
            s_t = smallp.tile([128, 1], F32, tag="s")
            nc.vector.tensor_add(s_t, s_hs[0], s_hs[1])
            sc_t = smallp.tile([128, 1], F32, tag="sc")
            nc.vector.reciprocal(sc_t, s_t)
            sc2_t = smallp.tile([128, 1], F32, tag="sc2")
            nc.vector.tensor_scalar_mul(sc2_t, sc_t, r_ts[bt][:, e : e + 1])
            if e == 0:
                nc.vector.tensor_scalar_mul(accs[bt], exp_t, sc2_t)
            elif e == E - 1:
                # split the combine by halves to shorten the writeback chain
                for oc in range(2):
                    osl = slice(oc * 512, (oc + 1) * 512)
                    nc.vector.scalar_tensor_tensor(
                        accs[bt][:, osl], exp_t[:, osl], sc2_t, accs[bt][:, osl],
                        op0=ALU.mult, op1=ALU.add,
                    )
                # writeback: full-partition-span pieces (small spans run at
                # half DMA rate); the final tile fans out over 8 queues
                r0 = bt * 128
                if bt == BT - 1:
                    for q in range(8):
                        nc.sync.dma_start(
                            out=out_d[r0 + (q % 2) * 64 : r0 + (q % 2) * 64 + 64,
                                      (q // 2) * 256 : (q // 2 + 1) * 256],
                            in_=accs[bt][(q % 2) * 64 : (q % 2) * 64 + 64,
                                         (q // 2) * 256 : (q // 2 + 1) * 256],
                        )
                else:
                    for q in range(4):
                        nc.sync.dma_start(
                            out=out_d[r0 : r0 + 128, q * 256 : (q + 1) * 256],
                            in_=accs[bt][:, q * 256 : (q + 1) * 256],
                        )
            else:
                nc.vector.scalar_tensor_tensor(
                    accs[bt], exp_t, sc2_t, accs[bt], op0=ALU.mult, op1=ALU.add
                )

        # ---- emission order: per-block waves while x streams in ----
        ble0_t = None
        if add_bl:
            ble0_t = smallp.tile([1, OUT_F], MDT, tag="ble", bufs=2)
            nc.sync.dma_start(out=ble0_t, in_=bl_d[0:1, :])
        for k in range(NBC):
            routing_wave(k)
            for bt in range(4 * k, 4 * k + 4):
                expert_bt(0, bt, w0_t, ble0_t)

        w_tiles = {1: w1_t}
        for e in range(1, E):
            if e + 1 < E:
                # prefetch next expert's weights so its transfers sit in the
                # queues a full expert (~55us of PE work) ahead of use
                w_tiles[e + 1] = dma_w(e + 1)
            w_t = w_tiles.pop(e)
            ble_t = None
            if add_bl:
                ble_t = smallp.tile([1, OUT_F], MDT, tag="ble", bufs=2)
                nc.sync.dma_start(out=ble_t, in_=bl_d[e : e + 1, :])
            for bt in range(BT):
                expert_bt(e, bt, w_t, ble_t)

    nc.compile()
    return nc


def make_core_inputs(x, Wd, bd, Wl, bl, core, add_bd, add_bl, mmdt: str = MMDT):
    import ml_dtypes

    ndt = {"bf16": ml_dtypes.bfloat16, "f16": np.float16}[mmdt]
    lo, hi = core * E, (core + 1) * E
    m = {
        "xT": np.ascontiguousarray(x.T.astype(ndt)),
        "wdT": np.ascontiguousarray(Wd.T.astype(ndt)),
        "wl": np.ascontiguousarray(Wl[lo:hi].transpose(0, 2, 1).astype(ndt)),
        "amat": np.ascontiguousarray(A_FULL[:, lo:hi]),
        "bmat": np.ascontiguousarray(B_FULL[:, lo:hi]),
    }
    if add_bd:
        m["bd"] = np.ascontiguousarray(bd.astype(ndt)).reshape(1, N_NODES)
    if add_bl:
        m["bl"] = np.ascontiguousarray(bl[lo:hi].astype(ndt))
    return m


_NC_CACHE = {}


def _get_nc(add_bd, add_bl, mmdt: str = MMDT):
    key = (add_bd, add_bl, mmdt)
    if key not in _NC_CACHE:
        _NC_CACHE[key] = build_nc(add_bd, add_bl, mmdt)
    return _NC_CACHE[key]


def run_spmd(x, Wd, bd, Wl, bl, trace=False, mmdt: str = MMDT):
    add_bd = bool(np.any(bd))
    add_bl = bool(np.any(bl))
    nc = _get_nc(add_bd, add_bl, mmdt)
    in_maps = [
        make_core_inputs(x, Wd, bd, Wl, bl, c, add_bd, add_bl, mmdt)
        for c in range(N_CORES)
    ]
    res = run_bass_kernel_spmd(nc, in_maps, core_ids=list(range(N_CORES)), trace=trace)
    partials = np.stack([r["out"] for r in res.results], axis=0)
    out = partials.sum(axis=0, dtype=np.float64).astype(np.float32)
    return out, res


def kernel(x, Wd, bd, Wl, bl):
    out, _ = run_spmd(
        np.asarray(x), np.asarray(Wd), np.asarray(bd), np.asarray(Wl), np.asarray(bl)
    )
    return out


# revision 18
# speedup vs baseline: 1.0584x; 1.0109x over previous
"""DNDF tree (soft decision tree / dense MoE) kernel for Trainium2.

Full computation (reference):
    dprob  = sigmoid(x @ Wd.T + bd)                 [B, 63]
    routing[b, l] = prod_d (pos ? dprob[idx] : 1 - dprob[idx])   [B, 64]
    leaves = softmax(einsum('bi,loi->blo', x, Wl) + bl, axis=-1) [B, 64, O]
    out    = einsum('bl,blo->bo', routing, leaves)  [B, O]

Sharding: expert-parallel over the 64 leaves: core c owns leaves
8c..8c+7, computes partial = sum_{l in core} routing[:, l] * leaves[:, l, :]
over the FULL batch; the host sums the 8 per-core partials.

Routing trick (no gathers): with z = x@Wd.T + bd,
    log p      = -softplus(-z),   log(1-p) = -softplus(z)
    log routing[b,l] = -( softplus(-z) @ A + softplus(z) @ B )[b, l]
where A[n,l]=1 iff leaf l visits node n on the sigmoid branch and
B[n,l]=1 iff on the (1-sigmoid) branch.  Two small matmuls + exp.
softplus(t) = Ln(Exp(t)+1) — computed with the one ACT table set that
has both exp and ln (Softplus has no table on cayman).

float16 matmuls: 10-bit mantissa keeps end-to-end error ~3e-4 while
streaming 1 col/cycle on the PE with FWL weight loads.  (fp8 DoubleRow
was evaluated and rejected: e4m3 quantization alone gives rel_err
2.3e-2 > the 2e-2 gate, and any 2-slot correction scheme lands at f16
cost with worse per-slot throughput.)

Schedule (from perfetto analysis of the 516us baseline):
  - PE warm-up runs on a memset tile (no DMA dependency) so the HAM
    clock-gate reaches 8/8 at ~1us and the routing matmuls never run at
    the 1.2-1.6 GHz cold rate (was ~16us of cold tax + 11.5us of gaps).
  - x is DMA'd batch-block-major and interleaved with expert 0's
    weights so each 512-sample block's routing (z matmuls + softplus +
    leaf-product) and expert-0 tiles start as soon as that block lands.
  - Output writeback uses full 4 KiB DRAM lines; the last batch tile
    fans out over 16 queues to cut the drain tail.
"""

import numpy as np
import sys

for _p in ("/opt/trn_rl_repo", "/opt/pypackages"):
    if _p not in sys.path:
        sys.path.append(_p)

import concourse.bass as bass  # noqa: E402,F401
import concourse.bacc as bacc  # noqa: E402
import concourse.tile as tile  # noqa: E402
from concourse import mybir  # noqa: E402
from concourse.bass_utils import run_bass_kernel_spmd  # noqa: E402

TREE_DEPTH = 6
IN_F = 1024
OUT_F = 1024
BATCH = 2048
N_LEAVES = 64
N_NODES = 63
N_CORES = 8
E = N_LEAVES // N_CORES  # experts per core = 8
IC = IN_F // 128  # contraction chunks = 8
BT = BATCH // 128  # batch tiles = 16
NBC = BATCH // 512  # 512-sample batch blocks = 4
F32 = mybir.dt.float32
BF16 = mybir.dt.bfloat16
F16 = mybir.dt.float16
F8 = mybir.dt.float8e4
AF = mybir.ActivationFunctionType
ALU = mybir.AluOpType

# "mixed": expert-GEMM K-chunks 0-3 in f16, chunks 4-7 as two e4m3
# DoubleRow matmuls (2 chunks each at 0.5 cyc/row).  CPU-sim rel_err
# 1.63e-2 vs the 2e-2 gate (f16: 2.8e-4); 22% less PE time per tile.
# Scales: x/8 and W*8 keep both e4m3 operands in the normal range and
# cancel in the product, so fp8 chunks accumulate directly with f16 ones.
MMDT = "f16"
X8SCALE = 8.0
W8SCALE = 8.0
WARM_N = 43  # warm-up matmuls bridging engine start -> first x block (~20us)
TAPER_N = 12  # second warm burst bridging routing -> expert-0 weight arrival


def _tree_routes(depth):
    n_leaves = 2**depth
    idx = np.zeros((n_leaves, depth), dtype=np.int32)
    pos = np.zeros((n_leaves, depth), dtype=bool)
    for leaf in range(n_leaves):
        node, index = 0, leaf
        for d in range(depth):
            idx[leaf, d] = node
            pos[leaf, d] = index % 2 == 0
            node = node * 2 + 1 + index % 2
            index >>= 1
    return idx, pos


ROUTE_IDX, ROUTE_POS = _tree_routes(TREE_DEPTH)


def _selection_matrices():
    """A[n, l]=1 iff leaf l visits node n with the sigmoid branch; B for 1-sigmoid."""
    A = np.zeros((N_NODES, N_LEAVES), dtype=np.float32)
    B = np.zeros((N_NODES, N_LEAVES), dtype=np.float32)
    for leaf in range(N_LEAVES):
        for d in range(TREE_DEPTH):
            n = ROUTE_IDX[leaf, d]
            if ROUTE_POS[leaf, d]:
                A[n, leaf] = 1.0
            else:
                B[n, leaf] = 1.0
    return A, B


A_FULL, B_FULL = _selection_matrices()


def build_nc(add_bd: bool, add_bl: bool, mmdt: str = MMDT):
    """Build + compile the single-core Bass program (same NEFF on all cores)."""
    from contextlib import ExitStack

    MDT = {"bf16": BF16, "f16": F16}[mmdt]

    nc = bacc.Bacc("TRN2", target_bir_lowering=False, debug=False)

    xT_d = nc.dram_tensor("xT", [IN_F, BATCH], MDT, kind="ExternalInput")
    wdT_d = nc.dram_tensor("wdT", [IN_F, N_NODES], MDT, kind="ExternalInput")
    wl_d = nc.dram_tensor("wl", [E, IN_F, OUT_F], MDT, kind="ExternalInput")
    a_d = nc.dram_tensor("amat", [N_NODES, E], F32, kind="ExternalInput")
    b_d = nc.dram_tensor("bmat", [N_NODES, E], F32, kind="ExternalInput")
    bd_d = bl_d = None
    if add_bd:
        bd_d = nc.dram_tensor("bd", [1, N_NODES], MDT, kind="ExternalInput")
    if add_bl:
        bl_d = nc.dram_tensor("bl", [E, OUT_F], MDT, kind="ExternalInput")
    out_d = nc.dram_tensor("out", [BATCH, OUT_F], F32, kind="ExternalOutput")

    mm = lambda out, lhsT, rhs, start, stop: nc.tensor.matmul(  # noqa: E731
        out, lhsT, rhs, start=start, stop=stop
    )

    with ExitStack() as ctx:
        tc = ctx.enter_context(tile.TileContext(nc))
        consts = ctx.enter_context(tc.tile_pool(name="consts", bufs=1))
        xp = ctx.enter_context(tc.tile_pool(name="xp", bufs=1))
        wp = ctx.enter_context(tc.tile_pool(name="wp", bufs=3))
        accp = ctx.enter_context(tc.tile_pool(name="accp", bufs=BT))
        expp = ctx.enter_context(tc.tile_pool(name="expp", bufs=3))
        rpool = ctx.enter_context(tc.tile_pool(name="rpool", bufs=17))
        spp = ctx.enter_context(tc.tile_pool(name="spp", bufs=1))
        smallp = ctx.enter_context(tc.tile_pool(name="smallp", bufs=6))
        zps = ctx.enter_context(tc.tile_pool(name="zps", bufs=3, space="PSUM"))
        rps = ctx.enter_context(tc.tile_pool(name="rps", bufs=2, space="PSUM"))
        lps = ctx.enter_context(tc.tile_pool(name="lps", bufs=3, space="PSUM"))

        # ---- PE warm-up with no DMA dependency (memset-sourced tile).
        # Starts as soon as the engines are live (~8us), flips the HAM clock
        # gate to 8/8 before the first real matmul, and bridges the DMA
        # spin-up window so routing runs at the warm rate.
        warm_t = consts.tile([128, 512], MDT)
        nc.vector.memset(warm_t, 0.05)
        warm_ps = zps.tile([128, 512], F32, tag="z")
        for i in range(WARM_N):
            mm(warm_ps, warm_t[:, 0:128], warm_t, start=(i == 0), stop=(i == WARM_N - 1))
        junk_t = smallp.tile([128, 1], F32, tag="junk")
        nc.vector.reduce_max(junk_t, warm_ps, axis=mybir.AxisListType.X)

        # ---- streamed inputs.  x batch-block 0 goes out first, split 16
        # ways so every queue carries one 64 KiB piece and the block lands
        # ~3us after queue spin-up.
        x_t = xp.tile([128, IC, BATCH], MDT, tag="x", name="x0")

        def dma_x_block(k, nsplit=1):
            for c in range(IC):
                for s in range(nsplit):
                    r0 = c * 128 + s * (128 // nsplit)
                    r1 = r0 + 128 // nsplit
                    p0 = s * (128 // nsplit)
                    nc.sync.dma_start(
                        out=x_t[p0 : p0 + 128 // nsplit, c, k * 512 : (k + 1) * 512],
                        in_=xT_d[r0:r1, k * 512 : (k + 1) * 512],
                    )

        dma_x_block(0, nsplit=2)  # queues 0-15: one 64 KiB piece each

        wd_t = consts.tile([128, IC, N_NODES], MDT)  # tiny, queued 2nd
        for c in range(IC):
            nc.sync.dma_start(out=wd_t[:, c, :], in_=wdT_d[c * 128 : (c + 1) * 128, :])
        a_t = consts.tile([N_NODES, E], F32)
        nc.sync.dma_start(out=a_t, in_=a_d[:])
        b_t = consts.tile([N_NODES, E], F32)
        nc.sync.dma_start(out=b_t, in_=b_d[:])
        ones_t = bd_t = None
        if add_bd or add_bl:
            ones_t = consts.tile([1, 512], MDT)
            nc.vector.memset(ones_t, 1.0)
        if add_bd:
            bd_t = consts.tile([1, N_NODES], MDT)
            nc.sync.dma_start(out=bd_t, in_=bd_d[:])

        def dma_w(e, ocs=(0, 1), nsplit=1, w_t=None):
            if w_t is None:
                w_t = wp.tile([128, IC, OUT_F], MDT, tag="w", name=f"w_{e}")
            for oc in ocs:
                for c in range(IC):
                    for s in range(nsplit):
                        r0 = c * 128 + s * (128 // nsplit)
                        p0 = s * (128 // nsplit)
                        nc.sync.dma_start(
                            out=w_t[
                                p0 : p0 + 128 // nsplit, c,
                                oc * 512 : (oc + 1) * 512,
                            ],
                            in_=wl_d[
                                e, r0 : r0 + 128 // nsplit,
                                oc * 512 : (oc + 1) * 512,
                            ],
                        )
            return w_t

        # x blocks 1-3 next (routing consumes them first), then expert 0/1
        # weights behind them
        dma_x_block(1)
        dma_x_block(2)
        dma_x_block(3)
        w0_t = dma_w(0)
        w1_t = dma_w(1)  # prefetch: in queues before the expert stream begins

        sp_pos = spp.tile([N_NODES, BATCH], F32, tag="sp_pos")
        sp_neg = spp.tile([N_NODES, BATCH], F32, tag="sp_neg")
        r_ts = {}

        def routing_wave(k):
            """z matmuls + softplus for batch block k, then leaf products
            (r) for its four 128-sample tiles."""
            sl = slice(k * 512, (k + 1) * 512)
            z_ps = zps.tile([N_NODES, 512], F32, tag="z")
            for c in range(IC):
                mm(
                    z_ps,
                    wd_t[:, c, :],
                    x_t[:, c, sl],
                    start=(c == 0),
                    stop=(c == IC - 1 and not add_bd),
                )
            if add_bd:
                mm(z_ps, bd_t[0:1, :], ones_t[0:1, :], start=False, stop=True)
            # softplus(z) and softplus(-z) via exp + ln(x+1)
            nc.scalar.activation(sp_pos[:, sl], z_ps, AF.Exp, scale=1.0)
            nc.scalar.activation(sp_pos[:, sl], sp_pos[:, sl], AF.Ln, bias=1.0)
            # softplus(-z) = softplus(z) - z (exact); DVE is idle here
            nc.vector.tensor_sub(sp_neg[:, sl], sp_pos[:, sl], z_ps)
            for bt in range(4 * k, 4 * k + 4):
                r_ps = rps.tile([128, E], F32, tag="r_ps")
                nc.tensor.matmul(
                    r_ps,
                    sp_neg[:, bt * 128 : (bt + 1) * 128],
                    a_t,
                    start=True,
                    stop=False,
                )
                nc.tensor.matmul(
                    r_ps,
                    sp_pos[:, bt * 128 : (bt + 1) * 128],
                    b_t,
                    start=False,
                    stop=True,
                )
                r_t = rpool.tile([128, E], F32, tag="r", name=f"r_{bt}")
                nc.scalar.activation(r_t, r_ps, AF.Exp, scale=-1.0)
                r_ts[bt] = r_t

        accs = [
            accp.tile([128, OUT_F], F32, tag="acc", name=f"acc_{bt}")
            for bt in range(BT)
        ]

        def expert_bt(e, bt, w_t, ble_t=None):
            """One batch tile of one expert: 16 matmuls, exp eviction with
            row-sum accumulators, softmax scale, routing-weighted combine.
            Expert 0 initializes the accumulator (mul), others add into it —
            every expert has the same engine mix, so no engine spikes."""
            exp_t = expp.tile([128, OUT_F], F32, tag="exp", name="exp_t")
            s_hs = []
            for oc in range(2):
                lp = lps.tile([128, 512], F32, tag="lp")
                for c in range(IC):
                    mm(
                        lp,
                        x_t[:, c, bt * 128 : (bt + 1) * 128],
                        w_t[:, c, oc * 512 : (oc + 1) * 512],
                        start=(c == 0),
                        stop=(c == IC - 1 and not add_bl),
                    )
                if add_bl:
                    mm(
                        lp,
                        ones_t[0:1, 0:128],
                        ble_t[0:1, oc * 512 : (oc + 1) * 512],
                        start=False,
                        stop=True,
                    )
                sh = smallp.tile([128, 1], F32, tag=f"sh{oc}", name=f"sh{oc}")
                nc.scalar.activation(
                    exp_t[:, oc * 512 : (oc + 1) * 512], lp, AF.Exp, accum_out=sh
                )
                s_hs.append(sh)
            s_t = smallp.tile([128, 1], F32, tag="s")
            nc.vector.tensor_add(s_t, s_hs[0], s_hs[1])
            sc_t = smallp.tile([128, 1], F32, tag="sc")
            nc.vector.reciprocal(sc_t, s_t)
            sc2_t = smallp.tile([128, 1], F32, tag="sc2")
            nc.vector.tensor_scalar_mul(sc2_t, sc_t, r_ts[bt][:, e : e + 1])
            if e == 0:
                nc.vector.tensor_scalar_mul(accs[bt], exp_t, sc2_t)
            elif e == E - 1:
                # split the combine by halves to shorten the writeback chain
                for oc in range(2):
                    osl = slice(oc * 512, (oc + 1) * 512)
                    nc.vector.scalar_tensor_tensor(
                        accs[bt][:, osl], exp_t[:, osl], sc2_t, accs[bt][:, osl],
                        op0=ALU.mult, op1=ALU.add,
                    )
                # writeback over 4 queues (full 128-partition span: smaller
                # spans measured at proportionally lower DMA rate)
                r0 = bt * 128
                for q in range(4):
                    nc.sync.dma_start(
                        out=out_d[r0 : r0 + 128, q * 256 : (q + 1) * 256],
                        in_=accs[bt][:, q * 256 : (q + 1) * 256],
                    )
            else:
                nc.vector.scalar_tensor_tensor(
                    accs[bt], exp_t, sc2_t, accs[bt], op0=ALU.mult, op1=ALU.add
                )

        # ---- emission order: routing waves track x's block arrival; a short
        # second warm burst bridges to expert 0's weight arrival so the PE
        # never idles >3.4us (the HAM re-throttle window) ----
        ble0_t = None
        if add_bl:
            ble0_t = smallp.tile([1, OUT_F], MDT, tag="ble", bufs=2)
            nc.sync.dma_start(out=ble0_t, in_=bl_d[0:1, :])
        for k in range(NBC):
            routing_wave(k)
        taper_ps = zps.tile([128, 512], F32, tag="z")
        for i in range(TAPER_N):
            mm(taper_ps, warm_t[:, 0:128], warm_t, start=(i == 0), stop=(i == TAPER_N - 1))
        junk2_t = smallp.tile([128, 1], F32, tag="junk")
        nc.vector.reduce_max(junk2_t, taper_ps, axis=mybir.AxisListType.X)
        for bt in range(BT):
            expert_bt(0, bt, w0_t, ble0_t)

        w_tiles = {1: w1_t}
        for e in range(1, E):
            if e + 1 < E:
                # prefetch next expert's weights so its transfers sit in the
                # queues a full expert (~55us of PE work) ahead of use
                w_tiles[e + 1] = dma_w(e + 1)
            w_t = w_tiles.pop(e)
            ble_t = None
            if add_bl:
                ble_t = smallp.tile([1, OUT_F], MDT, tag="ble", bufs=2)
                nc.sync.dma_start(out=ble_t, in_=bl_d[e : e + 1, :])
            for bt in range(BT):
                expert_bt(e, bt, w_t, ble_t)

    nc.compile()
    return nc


def make_core_inputs(x, Wd, bd, Wl, bl, core, add_bd, add_bl, mmdt: str = MMDT):
    import ml_dtypes

    ndt = {"bf16": ml_dtypes.bfloat16, "f16": np.float16}[mmdt]
    lo, hi = core * E, (core + 1) * E
    m = {
        "xT": np.ascontiguousarray(x.T.astype(ndt)),
        "wdT": np.ascontiguousarray(Wd.T.astype(ndt)),
        "wl": np.ascontiguousarray(Wl[lo:hi].transpose(0, 2, 1).astype(ndt)),
        "amat": np.ascontiguousarray(A_FULL[:, lo:hi]),
        "bmat": np.ascontiguousarray(B_FULL[:, lo:hi]),
    }
    if add_bd:
        m["bd"] = np.ascontiguousarray(bd.astype(ndt)).reshape(1, N_NODES)
    if add_bl:
        m["bl"] = np.ascontiguousarray(bl[lo:hi].astype(ndt))
    return m


_NC_CACHE = {}


def _get_nc(add_bd, add_bl, mmdt: str = MMDT):
    key = (add_bd, add_bl, mmdt)
    if key not in _NC_CACHE:
        _NC_CACHE[key] = build_nc(add_bd, add_bl, mmdt)
    return _NC_CACHE[key]


def run_spmd(x, Wd, bd, Wl, bl, trace=False, mmdt: str = MMDT):
    add_bd = bool(np.any(bd))
    add_bl = bool(np.any(bl))
    nc = _get_nc(add_bd, add_bl, mmdt)
    in_maps = [
        make_core_inputs(x, Wd, bd, Wl, bl, c, add_bd, add_bl, mmdt)
        for c in range(N_CORES)
    ]
    res = run_bass_kernel_spmd(nc, in_maps, core_ids=list(range(N_CORES)), trace=trace)
    partials = np.stack([r["out"] for r in res.results], axis=0)
    out = partials.sum(axis=0, dtype=np.float64).astype(np.float32)
    return out, res


def kernel(x, Wd, bd, Wl, bl):
    out, _ = run_spmd(
        np.asarray(x), np.asarray(Wd), np.asarray(bd), np.asarray(Wl), np.asarray(bl)
    )
    return out


# revision 24
# speedup vs baseline: 1.2890x; 1.2179x over previous
"""DNDF tree (soft decision tree / dense MoE) kernel for Trainium2.

Full computation (reference):
    dprob  = sigmoid(x @ Wd.T + bd)                 [B, 63]
    routing[b, l] = prod_d (pos ? dprob[idx] : 1 - dprob[idx])   [B, 64]
    leaves = softmax(einsum('bi,loi->blo', x, Wl) + bl, axis=-1) [B, 64, O]
    out    = einsum('bl,blo->bo', routing, leaves)  [B, O]

Sharding: expert-parallel over the 64 leaves: core c owns leaves
8c..8c+7, computes partial = sum_{l in core} routing[:, l] * leaves[:, l, :]
over the FULL batch; the host sums the 8 per-core partials.

Routing trick (no gathers): with z = x@Wd.T + bd,
    log p      = -softplus(-z),   log(1-p) = -softplus(z)
    log routing[b,l] = -( softplus(-z) @ A + softplus(z) @ B )[b, l]
where A[n,l]=1 iff leaf l visits node n on the sigmoid branch and
B[n,l]=1 iff on the (1-sigmoid) branch.  Two small matmuls + exp.
softplus(t) = Ln(Exp(t)+1) — computed with the one ACT table set that
has both exp and ln (Softplus has no table on cayman).

float16 matmuls: 10-bit mantissa keeps end-to-end error ~3e-4 while
streaming 1 col/cycle on the PE with FWL weight loads.  (fp8 DoubleRow
was evaluated and rejected: e4m3 quantization alone gives rel_err
2.3e-2 > the 2e-2 gate, and any 2-slot correction scheme lands at f16
cost with worse per-slot throughput.)

Schedule (from perfetto analysis of the 516us baseline):
  - PE warm-up runs on a memset tile (no DMA dependency) so the HAM
    clock-gate reaches 8/8 at ~1us and the routing matmuls never run at
    the 1.2-1.6 GHz cold rate (was ~16us of cold tax + 11.5us of gaps).
  - x is DMA'd batch-block-major and interleaved with expert 0's
    weights so each 512-sample block's routing (z matmuls + softplus +
    leaf-product) and expert-0 tiles start as soon as that block lands.
  - Output writeback uses full 4 KiB DRAM lines; the last batch tile
    fans out over 16 queues to cut the drain tail.
"""

import numpy as np
import sys

for _p in ("/opt/trn_rl_repo", "/opt/pypackages"):
    if _p not in sys.path:
        sys.path.append(_p)

import concourse.bass as bass  # noqa: E402,F401
import concourse.bacc as bacc  # noqa: E402
import concourse.tile as tile  # noqa: E402
from concourse import mybir  # noqa: E402
from concourse.bass_utils import run_bass_kernel_spmd  # noqa: E402

TREE_DEPTH = 6
IN_F = 1024
OUT_F = 1024
BATCH = 2048
N_LEAVES = 64
N_NODES = 63
N_CORES = 8
E = N_LEAVES // N_CORES  # experts per core = 8
IC = IN_F // 128  # contraction chunks = 8
BT = BATCH // 128  # batch tiles = 16
NBC = BATCH // 512  # 512-sample batch blocks = 4
F32 = mybir.dt.float32
BF16 = mybir.dt.bfloat16
F16 = mybir.dt.float16
F8 = mybir.dt.float8e4
AF = mybir.ActivationFunctionType
ALU = mybir.AluOpType

# "mixed": expert-GEMM K-chunks 0-3 in f16, chunks 4-7 as two e4m3
# DoubleRow matmuls (2 chunks each at 0.5 cyc/row).  CPU-sim rel_err
# 1.63e-2 vs the 2e-2 gate (f16: 2.8e-4); 22% less PE time per tile.
# Scales: x/8 and W*8 keep both e4m3 operands in the normal range and
# cancel in the product, so fp8 chunks accumulate directly with f16 ones.
MMDT = "mixed"
X8SCALE = 8.0
W8SCALE = 8.0
WARM_N = 43  # warm-up matmuls bridging engine start -> first x block (~20us)
TAPER_N = 12  # second warm burst bridging routing -> expert-0 weight arrival


def _tree_routes(depth):
    n_leaves = 2**depth
    idx = np.zeros((n_leaves, depth), dtype=np.int32)
    pos = np.zeros((n_leaves, depth), dtype=bool)
    for leaf in range(n_leaves):
        node, index = 0, leaf
        for d in range(depth):
            idx[leaf, d] = node
            pos[leaf, d] = index % 2 == 0
            node = node * 2 + 1 + index % 2
            index >>= 1
    return idx, pos


ROUTE_IDX, ROUTE_POS = _tree_routes(TREE_DEPTH)


def _selection_matrices():
    """A[n, l]=1 iff leaf l visits node n with the sigmoid branch; B for 1-sigmoid."""
    A = np.zeros((N_NODES, N_LEAVES), dtype=np.float32)
    B = np.zeros((N_NODES, N_LEAVES), dtype=np.float32)
    for leaf in range(N_LEAVES):
        for d in range(TREE_DEPTH):
            n = ROUTE_IDX[leaf, d]
            if ROUTE_POS[leaf, d]:
                A[n, leaf] = 1.0
            else:
                B[n, leaf] = 1.0
    return A, B


A_FULL, B_FULL = _selection_matrices()


def build_nc(add_bd: bool, add_bl: bool, mmdt: str = MMDT):
    """Build + compile the single-core Bass program (same NEFF on all cores)."""
    from contextlib import ExitStack

    mixed = mmdt == "mixed"
    MDT = {"bf16": BF16, "f16": F16, "mixed": F16}[mmdt]
    ICF = 4 if mixed else IC  # f16 contraction chunks in the expert GEMM

    nc = bacc.Bacc("TRN2", target_bir_lowering=False, debug=False)

    xT_d = nc.dram_tensor("xT", [IN_F, BATCH], MDT, kind="ExternalInput")
    wdT_d = nc.dram_tensor("wdT", [IN_F, N_NODES], MDT, kind="ExternalInput")
    wl_d = nc.dram_tensor("wl", [E, ICF * 128, OUT_F], MDT, kind="ExternalInput")
    xT8_d = wl8_d = None
    if mixed:
        xT8_d = nc.dram_tensor("xT8", [512, BATCH], F8, kind="ExternalInput")
        wl8_d = nc.dram_tensor("wl8", [E, 512, OUT_F], F8, kind="ExternalInput")
    a_d = nc.dram_tensor("amat", [N_NODES, E], F32, kind="ExternalInput")
    b_d = nc.dram_tensor("bmat", [N_NODES, E], F32, kind="ExternalInput")
    bd_d = bl_d = None
    if add_bd:
        bd_d = nc.dram_tensor("bd", [1, N_NODES], MDT, kind="ExternalInput")
    if add_bl:
        bl_d = nc.dram_tensor("bl", [E, OUT_F], MDT, kind="ExternalInput")
    out_d = nc.dram_tensor("out", [BATCH, OUT_F], F32, kind="ExternalOutput")

    mm = lambda out, lhsT, rhs, start, stop: nc.tensor.matmul(  # noqa: E731
        out, lhsT, rhs, start=start, stop=stop
    )

    with ExitStack() as ctx:
        tc = ctx.enter_context(tile.TileContext(nc))
        consts = ctx.enter_context(tc.tile_pool(name="consts", bufs=1))
        xp = ctx.enter_context(tc.tile_pool(name="xp", bufs=1))
        wp = ctx.enter_context(tc.tile_pool(name="wp", bufs=3))
        accp = ctx.enter_context(tc.tile_pool(name="accp", bufs=BT))
        expp = ctx.enter_context(tc.tile_pool(name="expp", bufs=3))
        rpool = ctx.enter_context(tc.tile_pool(name="rpool", bufs=17))
        spp = ctx.enter_context(tc.tile_pool(name="spp", bufs=1))
        smallp = ctx.enter_context(tc.tile_pool(name="smallp", bufs=6))
        zps = ctx.enter_context(tc.tile_pool(name="zps", bufs=3, space="PSUM"))
        rps = ctx.enter_context(tc.tile_pool(name="rps", bufs=2, space="PSUM"))
        lps = ctx.enter_context(tc.tile_pool(name="lps", bufs=3, space="PSUM"))

        # ---- PE warm-up with no DMA dependency (memset-sourced tile).
        # Starts as soon as the engines are live (~8us), flips the HAM clock
        # gate to 8/8 before the first real matmul, and bridges the DMA
        # spin-up window so routing runs at the warm rate.
        warm_t = consts.tile([128, 512], MDT)
        nc.vector.memset(warm_t, 0.05)
        warm_ps = zps.tile([128, 512], F32, tag="z")
        for i in range(WARM_N):
            mm(warm_ps, warm_t[:, 0:128], warm_t, start=(i == 0), stop=(i == WARM_N - 1))
        junk_t = smallp.tile([128, 1], F32, tag="junk")
        nc.vector.reduce_max(junk_t, warm_ps, axis=mybir.AxisListType.X)

        # ---- streamed inputs.  x batch-block 0 goes out first, split 16
        # ways so every queue carries one 64 KiB piece and the block lands
        # ~3us after queue spin-up.
        x_t = xp.tile([128, IC, BATCH], MDT, tag="x", name="x0")
        x8_t = None
        if mixed:
            x8_t = xp.tile([128, 4, BATCH], F8, tag="x8", name="x8")

        def dma_x_block(k, nsplit=1):
            for c in range(IC):
                for s in range(nsplit):
                    r0 = c * 128 + s * (128 // nsplit)
                    r1 = r0 + 128 // nsplit
                    p0 = s * (128 // nsplit)
                    nc.sync.dma_start(
                        out=x_t[p0 : p0 + 128 // nsplit, c, k * 512 : (k + 1) * 512],
                        in_=xT_d[r0:r1, k * 512 : (k + 1) * 512],
                    )
            if mixed:
                for c in range(4):
                    nc.sync.dma_start(
                        out=x8_t[:, c, k * 512 : (k + 1) * 512],
                        in_=xT8_d[c * 128 : (c + 1) * 128, k * 512 : (k + 1) * 512],
                    )

        dma_x_block(0, nsplit=2)  # queues 0-15: one 64 KiB piece each

        wd_t = consts.tile([128, IC, N_NODES], MDT)  # tiny, queued 2nd
        for c in range(IC):
            nc.sync.dma_start(out=wd_t[:, c, :], in_=wdT_d[c * 128 : (c + 1) * 128, :])
        a_t = consts.tile([N_NODES, E], F32)
        nc.sync.dma_start(out=a_t, in_=a_d[:])
        b_t = consts.tile([N_NODES, E], F32)
        nc.sync.dma_start(out=b_t, in_=b_d[:])
        ones_t = bd_t = None
        if add_bd or add_bl:
            ones_t = consts.tile([1, 512], MDT)
            nc.vector.memset(ones_t, 1.0)
        if add_bd:
            bd_t = consts.tile([1, N_NODES], MDT)
            nc.sync.dma_start(out=bd_t, in_=bd_d[:])

        def dma_w(e, ocs=(0, 1), nsplit=1, w_t=None):
            if w_t is None:
                wf = wp.tile([128, ICF, OUT_F], MDT, tag="w", name=f"w_{e}")
                w8 = None
                if mixed:
                    w8 = wp.tile([128, 4, OUT_F], F8, tag="w8", name=f"w8_{e}")
                w_t = (wf, w8)
            wf, w8 = w_t
            for oc in ocs:
                for c in range(ICF):
                    for s in range(nsplit):
                        r0 = c * 128 + s * (128 // nsplit)
                        p0 = s * (128 // nsplit)
                        nc.sync.dma_start(
                            out=wf[
                                p0 : p0 + 128 // nsplit, c,
                                oc * 512 : (oc + 1) * 512,
                            ],
                            in_=wl_d[
                                e, r0 : r0 + 128 // nsplit,
                                oc * 512 : (oc + 1) * 512,
                            ],
                        )
                if mixed:
                    for c in range(4):
                        nc.sync.dma_start(
                            out=w8[:, c, oc * 512 : (oc + 1) * 512],
                            in_=wl8_d[
                                e, c * 128 : (c + 1) * 128, oc * 512 : (oc + 1) * 512
                            ],
                        )
            return w_t

        # x blocks 1-3 next (routing consumes them first), then expert 0/1
        # weights behind them
        dma_x_block(1)
        dma_x_block(2)
        dma_x_block(3)
        w0_t = dma_w(0)
        w1_t = dma_w(1)  # prefetch: in queues before the expert stream begins

        sp_pos = spp.tile([N_NODES, BATCH], F32, tag="sp_pos")
        sp_neg = spp.tile([N_NODES, BATCH], F32, tag="sp_neg")
        r_ts = {}

        def routing_wave(k):
            """z matmuls + softplus for batch block k, then leaf products
            (r) for its four 128-sample tiles."""
            sl = slice(k * 512, (k + 1) * 512)
            z_ps = zps.tile([N_NODES, 512], F32, tag="z")
            for c in range(IC):
                mm(
                    z_ps,
                    wd_t[:, c, :],
                    x_t[:, c, sl],
                    start=(c == 0),
                    stop=(c == IC - 1 and not add_bd),
                )
            if add_bd:
                mm(z_ps, bd_t[0:1, :], ones_t[0:1, :], start=False, stop=True)
            # softplus(z) and softplus(-z) via exp + ln(x+1)
            nc.scalar.activation(sp_pos[:, sl], z_ps, AF.Exp, scale=1.0)
            nc.scalar.activation(sp_pos[:, sl], sp_pos[:, sl], AF.Ln, bias=1.0)
            # softplus(-z) = softplus(z) - z (exact); DVE is idle here
            nc.vector.tensor_sub(sp_neg[:, sl], sp_pos[:, sl], z_ps)
            for bt in range(4 * k, 4 * k + 4):
                r_ps = rps.tile([128, E], F32, tag="r_ps")
                nc.tensor.matmul(
                    r_ps,
                    sp_neg[:, bt * 128 : (bt + 1) * 128],
                    a_t,
                    start=True,
                    stop=False,
                )
                nc.tensor.matmul(
                    r_ps,
                    sp_pos[:, bt * 128 : (bt + 1) * 128],
                    b_t,
                    start=False,
                    stop=True,
                )
                r_t = rpool.tile([128, E], F32, tag="r", name=f"r_{bt}")
                nc.scalar.activation(r_t, r_ps, AF.Exp, scale=-1.0)
                r_ts[bt] = r_t

        accs = [
            accp.tile([128, OUT_F], F32, tag="acc", name=f"acc_{bt}")
            for bt in range(BT)
        ]

        def expert_bt(e, bt, w_t, ble_t=None):
            """One batch tile of one expert: 16 matmuls, exp eviction with
            row-sum accumulators, softmax scale, routing-weighted combine.
            Expert 0 initializes the accumulator (mul), others add into it —
            every expert has the same engine mix, so no engine spikes."""
            exp_t = expp.tile([128, OUT_F], F32, tag="exp", name="exp_t")
            wf, w8 = w_t
            s_hs = []
            for oc in range(2):
                lp = lps.tile([128, 512], F32, tag="lp")
                for c in range(ICF):
                    mm(
                        lp,
                        x_t[:, c, bt * 128 : (bt + 1) * 128],
                        wf[:, c, oc * 512 : (oc + 1) * 512],
                        start=(c == 0),
                        stop=(not mixed and c == IC - 1 and not add_bl),
                    )
                if mixed:
                    for cp in range(2):
                        nc.tensor.matmul(
                            lp,
                            x8_t[:, 2 * cp : 2 * cp + 2, bt * 128 : (bt + 1) * 128],
                            w8[:, 2 * cp : 2 * cp + 2, oc * 512 : (oc + 1) * 512],
                            start=False,
                            stop=(cp == 1 and not add_bl),
                            perf_mode=mybir.MatmulPerfMode.DoubleRow,
                        )
                if add_bl:
                    mm(
                        lp,
                        ones_t[0:1, 0:128],
                        ble_t[0:1, oc * 512 : (oc + 1) * 512],
                        start=False,
                        stop=True,
                    )
                sh = smallp.tile([128, 1], F32, tag=f"sh{oc}", name=f"sh{oc}")
                nc.scalar.activation(
                    exp_t[:, oc * 512 : (oc + 1) * 512], lp, AF.Exp, accum_out=sh
                )
                s_hs.append(sh)
            s_t = smallp.tile([128, 1], F32, tag="s")
            nc.vector.tensor_add(s_t, s_hs[0], s_hs[1])
            sc_t = smallp.tile([128, 1], F32, tag="sc")
            nc.vector.reciprocal(sc_t, s_t)
            sc2_t = smallp.tile([128, 1], F32, tag="sc2")
            nc.vector.tensor_scalar_mul(sc2_t, sc_t, r_ts[bt][:, e : e + 1])
            if e == 0:
                nc.vector.tensor_scalar_mul(accs[bt], exp_t, sc2_t)
            elif e == E - 1:
                # split the combine by halves to shorten the writeback chain
                for oc in range(2):
                    osl = slice(oc * 512, (oc + 1) * 512)
                    nc.vector.scalar_tensor_tensor(
                        accs[bt][:, osl], exp_t[:, osl], sc2_t, accs[bt][:, osl],
                        op0=ALU.mult, op1=ALU.add,
                    )
                # writeback over 4 queues (full 128-partition span: smaller
                # spans measured at proportionally lower DMA rate)
                r0 = bt * 128
                for q in range(4):
                    nc.sync.dma_start(
                        out=out_d[r0 : r0 + 128, q * 256 : (q + 1) * 256],
                        in_=accs[bt][:, q * 256 : (q + 1) * 256],
                    )
            else:
                nc.vector.scalar_tensor_tensor(
                    accs[bt], exp_t, sc2_t, accs[bt], op0=ALU.mult, op1=ALU.add
                )

        # ---- emission order: routing waves track x's block arrival; a short
        # second warm burst bridges to expert 0's weight arrival so the PE
        # never idles >3.4us (the HAM re-throttle window) ----
        ble0_t = None
        if add_bl:
            ble0_t = smallp.tile([1, OUT_F], MDT, tag="ble", bufs=2)
            nc.sync.dma_start(out=ble0_t, in_=bl_d[0:1, :])
        for k in range(NBC):
            routing_wave(k)
        taper_ps = zps.tile([128, 512], F32, tag="z")
        for i in range(TAPER_N):
            mm(taper_ps, warm_t[:, 0:128], warm_t, start=(i == 0), stop=(i == TAPER_N - 1))
        junk2_t = smallp.tile([128, 1], F32, tag="junk")
        nc.vector.reduce_max(junk2_t, taper_ps, axis=mybir.AxisListType.X)
        for bt in range(BT):
            expert_bt(0, bt, w0_t, ble0_t)

        w_tiles = {1: w1_t}
        for e in range(1, E):
            if e + 1 < E:
                # prefetch next expert's weights so its transfers sit in the
                # queues a full expert (~55us of PE work) ahead of use
                w_tiles[e + 1] = dma_w(e + 1)
            w_t = w_tiles.pop(e)
            ble_t = None
            if add_bl:
                ble_t = smallp.tile([1, OUT_F], MDT, tag="ble", bufs=2)
                nc.sync.dma_start(out=ble_t, in_=bl_d[e : e + 1, :])
            for bt in range(BT):
                expert_bt(e, bt, w_t, ble_t)

    nc.compile()
    return nc


def make_core_inputs(x, Wd, bd, Wl, bl, core, add_bd, add_bl, mmdt: str = MMDT):
    import ml_dtypes

    mixed = mmdt == "mixed"
    ndt = {"bf16": ml_dtypes.bfloat16, "f16": np.float16, "mixed": np.float16}[mmdt]
    f8 = ml_dtypes.float8_e4m3  # IEEE-style e4m3, max 240 — matches TRN FP8_EXP4
    lo, hi = core * E, (core + 1) * E
    xT = x.T
    wlT = Wl[lo:hi].transpose(0, 2, 1)  # [E, IN_F, OUT_F]
    m = {
        "xT": np.ascontiguousarray(xT.astype(ndt)),
        "wdT": np.ascontiguousarray(Wd.T.astype(ndt)),
        "amat": np.ascontiguousarray(A_FULL[:, lo:hi]),
        "bmat": np.ascontiguousarray(B_FULL[:, lo:hi]),
    }
    if mixed:
        m["wl"] = np.ascontiguousarray(wlT[:, :512, :].astype(ndt))
        m["xT8"] = np.ascontiguousarray((xT[512:] / X8SCALE).astype(f8))
        m["wl8"] = np.ascontiguousarray((wlT[:, 512:, :] * W8SCALE).astype(f8))
    else:
        m["wl"] = np.ascontiguousarray(wlT.astype(ndt))
    if add_bd:
        m["bd"] = np.ascontiguousarray(bd.astype(ndt)).reshape(1, N_NODES)
    if add_bl:
        m["bl"] = np.ascontiguousarray(bl[lo:hi].astype(ndt))
    return m


_NC_CACHE = {}


def _get_nc(add_bd, add_bl, mmdt: str = MMDT):
    key = (add_bd, add_bl, mmdt)
    if key not in _NC_CACHE:
        _NC_CACHE[key] = build_nc(add_bd, add_bl, mmdt)
    return _NC_CACHE[key]


def run_spmd(x, Wd, bd, Wl, bl, trace=False, mmdt: str = MMDT):
    add_bd = bool(np.any(bd))
    add_bl = bool(np.any(bl))
    nc = _get_nc(add_bd, add_bl, mmdt)
    in_maps = [
        make_core_inputs(x, Wd, bd, Wl, bl, c, add_bd, add_bl, mmdt)
        for c in range(N_CORES)
    ]
    res = run_bass_kernel_spmd(nc, in_maps, core_ids=list(range(N_CORES)), trace=trace)
    partials = np.stack([r["out"] for r in res.results], axis=0)
    out = partials.sum(axis=0, dtype=np.float64).astype(np.float32)
    return out, res


def kernel(x, Wd, bd, Wl, bl):
    out, _ = run_spmd(
        np.asarray(x), np.asarray(Wd), np.asarray(bd), np.asarray(Wl), np.asarray(bl)
    )
    return out
